# revision 30
# baseline (speedup 1.0000x reference)
"""Multi-head attention (B=2, L=2048, dim=1024, 16 heads) on 8 Trainium2 cores.

Sharding: 8 cores = 2 (batch) x 4 (head groups of 4 heads). Each core runs an
identical Bass program on its own slice (SPMD, no collectives); the host sums
the 4 per-head-group partial projection outputs per batch and adds the bias.

Per-core dataflow (bf16 matmul operands, fp32 PSUM accumulation):
  xT [1024, 2048]  (x[b] transposed, channel-major, bf16)
  V token-major [128 tok, 4 heads, 64+1] (ones column fused for the softmax
    denominator), qT/kT feature-major [128 (2 heads x 64d), 2048]
  ST[k, q] = kT.T @ qT    (K=64 contraction, head pairs row-packed in the PE)
  PT = exp(ST / 8)        (ScalarE, PSUM -> SBUF bf16)
  OT[d, q] += V.T @ PT    (M=65: row 64 accumulates the softmax denominator)
  OT_norm = OT * bcast(1/denom), out = OT_norm.T @ wpT

Scheduling: one flat software pipeline paced by the ScalarE exp stream.
x and w_qkv live in single wide SBUF tiles loaded by ONE dma_start per
512-token x wave / per w section (HWDGE issue on the sync sequencer costs
~0.6us per dma_start regardless of size, so consolidation is what gets the
first score matmul gated by only ~2 MB of DMA).  The attention loop is
software-pipelined (scores k+1 issue right after exp k; O-matmuls trail by 4
steps) and all other projection work is chopped into ~2-matmul micro-units
fed between attention steps at a budgeted rate, keeping the PE dense (no HAM
re-throttle) without pushing the next score matmul back by more than ~0.5us.
The per-(pair,qs) normalized outputs live in separate tiles so the out-proj
fillers never false-share with the current super's normalize writes, and the
softmax reciprocal uses the fast custom-DVE approximation (~51 ULP, fine for
a softmax denominator) instead of the ~6.5us iterative-divide reciprocal.
"""

import os
import numpy as np

B, L, C = 2, 2048, 1024
H, D = 16, 64
HL = 4            # heads per core (local)
PAIRS = 2         # head pairs per core
CT = C // 128     # 8 contraction tiles for the projections
TOK = L // 128    # 16 key-token tiles
QW = 512          # query tile width
QS = L // QW      # 4 query tiles
NCORES = 8
OTRAIL = 4        # O-accumulation matmuls trail the exp stream by this many kb

_cache = {}


def _build_nc():
    import concourse.bass as bass
    import concourse.mybir as mybir
    import concourse.tile as tile
    from concourse import bacc

    F32 = mybir.dt.float32
    BF16 = mybir.dt.bfloat16
    EXP = mybir.ActivationFunctionType.Exp

    nc = bacc.Bacc("TRN2", target_bir_lowering=False, debug=False,
                   num_devices=NCORES)

    xT = nc.declare_dram_parameter("xT", [C, L], BF16, isOutput=False)
    wT = nc.declare_dram_parameter("wT", [C, 3 * HL * D], BF16, isOutput=False)
    wpT = nc.declare_dram_parameter("wpT", [HL * D, C], BF16, isOutput=False)
    out = nc.declare_dram_parameter("out", [L, C], BF16, isOutput=True)
    # pair-0 half of the final out-proj chunk, streamed out early; the host
    # adds it into rows [L-QW, L) (the device-side add would serialize the
    # kernel tail behind the very last normalize)
    out2 = nc.declare_dram_parameter("out2", [2 * QW, C], BF16, isOutput=True)

    xTr = xT.rearrange("(c p) l -> p c l", p=128)
    wTr = wT.rearrange("(c p) m -> p c m", p=128)

    with tile.TileContext(nc) as tc:
        from contextlib import ExitStack
        with ExitStack() as ctx:
            xpool = ctx.enter_context(tc.tile_pool(name="x", bufs=1))
            wpool = ctx.enter_context(tc.tile_pool(name="w", bufs=1))
            wppool = ctx.enter_context(tc.tile_pool(name="wp", bufs=1))
            qkpool = ctx.enter_context(tc.tile_pool(name="qk", bufs=1))
            vpool = ctx.enter_context(tc.tile_pool(name="v", bufs=1))
            psS = ctx.enter_context(tc.tile_pool(name="psS", bufs=2, space="PSUM"))
            psO = ctx.enter_context(tc.tile_pool(name="psO", bufs=4, space="PSUM"))
            otpool = ctx.enter_context(tc.tile_pool(name="ot", bufs=1))
            ptpool = ctx.enter_context(tc.tile_pool(name="pt", bufs=8))
            rpool = ctx.enter_context(tc.tile_pool(name="r", bufs=2))
            rpool2 = ctx.enter_context(tc.tile_pool(name="r2", bufs=2))
            obpool = ctx.enter_context(tc.tile_pool(name="ob", bufs=4))

            # ---- warmup: exp table load + a PE matmul burst at t=0 so the
            # HAM clock gate is at full rate before the first projection
            warm = rpool.tile([128, 16], F32, name="warm", tag="warm")
            nc.vector.memset(warm[:, 0:8], 0.0)
            nc.scalar.activation(out=warm[:, 8:16], in_=warm[:, 0:8], func=EXP)
            warmb = rpool.tile([128, 512], BF16, name="warmb", tag="warmb")
            nc.vector.memset(warmb, 0.0)
            wps = psO.tile([64, 512], F32, name="warmps", tag="ot")
            for i in range(24):
                nc.tensor.matmul(wps, lhsT=warmb[:, 0:64], rhs=warmb,
                                 start=(i == 0), stop=(i == 23))

            # ---- input loads: one dma_start per x wave / w section ----------
            x_b = xpool.tile([128, CT, L], BF16, name="x", tag="x")
            w_b = wpool.tile([128, CT, 3 * HL * D], BF16, name="w", tag="w")

            def wsec(j):  # j: 0=q, 1=k, 2=v section of w_qkv
                nc.sync.dma_start(
                    out=w_b[:, :, j * HL * D:(j + 1) * HL * D],
                    in_=wTr[:, :, j * HL * D:(j + 1) * HL * D])

            def xwave(ns):
                nc.sync.dma_start(
                    out=x_b[:, :, QW * ns:QW * (ns + 1)],
                    in_=xTr[:, :, QW * ns:QW * (ns + 1)])

            # q+k sections are contiguous in w -> one DMA covers both
            nc.sync.dma_start(out=w_b[:, :, 0:2 * HL * D],
                              in_=wTr[:, :, 0:2 * HL * D])
            xwave(0)
            xwave(1)
            wsec(2)
            xwave(2)
            xwave(3)
            wp_t = []
            for p in range(PAIRS):
                t = wppool.tile([128, C], BF16, name=f"wp{p}", tag=f"wp{p}")
                nc.sync.dma_start(out=t, in_=wpT[2 * D * p:2 * D * (p + 1), :])
                wp_t.append(t)

            ones_s = vpool.tile([128, HL, 1], F32, name="ones_s", tag="ones_s")
            nc.vector.memset(ones_s, 1.0)

            # ---- projection building blocks (filler generators) -------------
            v_t = [None] * TOK

            def gen_v(t):
                ps = psO.tile([128, HL * D], F32, name="psv", tag="ot")
                for c0 in range(0, CT, 2):
                    for c in (c0, c0 + 1):
                        nc.tensor.matmul(
                            ps,
                            lhsT=x_b[:, c, 128 * t:128 * (t + 1)],
                            rhs=w_b[:, c, 2 * HL * D:3 * HL * D],
                            start=(c == 0), stop=(c == CT - 1),
                        )
                    yield
                vt = vpool.tile([128, HL, D + 1], BF16, name=f"v{t}", tag=f"v{t}")
                nc.vector.tensor_copy(out=vt[:, :, D:D + 1], in_=ones_s)
                nc.vector.tensor_copy(
                    out=vt[:, :, 0:D],
                    in_=ps.rearrange("p (h d) -> p h d", h=HL),
                )
                v_t[t] = vt
                yield

            qk_t = {}
            for p in range(PAIRS):
                for nm in ("q", "k"):
                    qk_t[(nm, p)] = qkpool.tile(
                        [128, L], BF16, name=f"{nm}{p}", tag=f"{nm}{p}")

            def gen_qk(nm, p, ns):
                j = 0 if nm == "q" else 1
                ps = psO.tile([128, QW], F32, name="psqk", tag="ot")
                for c0 in range(0, CT, 2):
                    for c in (c0, c0 + 1):
                        nc.tensor.matmul(
                            ps,
                            lhsT=w_b[:, c, j * HL * D + 128 * p:
                                     j * HL * D + 128 * (p + 1)],
                            rhs=x_b[:, c, QW * ns:QW * (ns + 1)],
                            start=(c == 0), stop=(c == CT - 1),
                        )
                    yield
                nc.vector.tensor_copy(
                    out=qk_t[(nm, p)][:, QW * ns:QW * (ns + 1)], in_=ps)
                yield

            # normalized attention outputs, one tile per (pair, qs) so the
            # out-proj fillers never false-share with later normalize writes
            ot_sb = [[otpool.tile([128, QW], BF16, name=f"otp{p}q{qs}",
                                  tag=f"otp{p}q{qs}")
                      for qs in range(QS)] for p in range(PAIRS)]

            def gen_proj(t, last=False, pairs=tuple(range(PAIRS)), dst=None,
                         dst_row=None):
                qs, tq = divmod(t, QW // 128)
                ob = obpool.tile([128, C], BF16, name="ob", tag="ob")
                if dst is None:
                    dst, dst_row = out, 128 * t
                for nh in range(C // QW):
                    ps = psO.tile([128, QW], F32, name="psp", tag="ot")
                    for j, p2 in enumerate(pairs):
                        nc.tensor.matmul(
                            ps,
                            lhsT=ot_sb[p2][qs][:, 128 * tq:128 * (tq + 1)],
                            rhs=wp_t[p2][:, QW * nh:QW * (nh + 1)],
                            start=(j == 0), stop=(j == len(pairs) - 1),
                        )
                    yield
                    if last and nh == 0:
                        # tail: ACT is idle by now, offload one copy there
                        nc.scalar.activation(
                            out=ob[:, QW * nh:QW * (nh + 1)], in_=ps,
                            func=mybir.ActivationFunctionType.Copy)
                    else:
                        nc.vector.tensor_copy(
                            out=ob[:, QW * nh:QW * (nh + 1)], in_=ps)
                    yield
                nc.sync.dma_start(out=dst[dst_row:dst_row + 128, :], in_=ob)

            # ---- prologue: only what gates the first exp ---------------------
            for _ in gen_qk("k", 0, 0):
                pass
            for _ in gen_qk("q", 0, 0):
                pass

            # ---- filler queue ------------------------------------------------
            # (gate, generator): generator may be advanced once the super
            # with index >= gate is being emitted.  Supers are p-major:
            # s = 4*p + qs.
            fillers = []

            def F(gate, g, min_kb=0):
                fillers.append((gate, min_kb, g))

            F(0, gen_qk("k", 0, 1))
            for t in range(0, 3):
                F(0, gen_v(t))
            F(0, gen_qk("k", 0, 2))
            for t in range(3, 6):
                F(0, gen_v(t))
            F(0, gen_qk("k", 0, 3))
            for t in range(6, 9):
                F(0, gen_v(t))
            F(0, gen_qk("q", 0, 1))
            for t in range(9, TOK):
                F(0, gen_v(t))
            for ns in range(QS):
                F(1, gen_qk("k", 1, ns))
            F(1, gen_qk("q", 1, 0))
            F(1, gen_qk("q", 0, 2))
            F(2, gen_qk("q", 0, 3))
            # proj units read the PREVIOUS super's normalize output, which
            # completes a few us into the gating super -- hold them until
            # mid-super so they never block the in-order PE stream
            for t in range(8, 12):
                F(3, gen_proj(t, pairs=(0,), dst=out2,
                              dst_row=128 * (t - 8)), min_kb=7)
            F(4, gen_qk("q", 1, 1))
            for t in range(12, TOK):
                F(4, gen_proj(t, pairs=(0,), dst=out2,
                              dst_row=QW + 128 * (t - 12)), min_kb=7)
            F(5, gen_qk("q", 1, 2))
            for t in range(0, 4):
                F(5, gen_proj(t), min_kb=7)
            F(6, gen_qk("q", 1, 3))
            for t in range(4, 8):
                F(6, gen_proj(t), min_kb=7)
            for t in range(8, 12):
                F(7, gen_proj(t, pairs=(1,)), min_kb=7)
            BUDGET = [8, 3, 2, 3, 3, 3, 3, 3]

            def pop_fillers(s, kb, budget):
                popped = 0
                while popped < budget and fillers:
                    gate, min_kb, g = fillers[0]
                    if gate > s or (gate == s and kb < min_kb):
                        break
                    try:
                        next(g)
                        popped += 1
                    except StopIteration:
                        fillers.pop(0)

            # ---- attention ---------------------------------------------------
            # software-pipelined: scores k+1 issue right after exp k; the
            # O-accumulation matmuls (which depend on exp output) trail by
            # OTRAIL steps so the PE never stalls waiting on ScalarE.
            for s in range(PAIRS * QS):
                p, qs = divmod(s, QS)
                kT = qk_t[("k", p)]
                qT = qk_t[("q", p)]
                ot_a = psO.tile([65, QW], F32, name="ot_a", tag="ot")
                ot_b = psO.tile([65, QW], F32, name="ot_b", tag="ot")
                sts = [None] * TOK
                pts = [None] * TOK

                def emit_s(kb):
                    st = psS.tile([128, 2 * QW], F32, name="st", tag="st")
                    nc.tensor.matmul(
                        st[:, 0:QW],
                        lhsT=kT[0:64, 128 * kb:128 * (kb + 1)],
                        rhs=qT[0:64, QW * qs:QW * (qs + 1)],
                        start=True, stop=True,
                    )
                    nc.tensor.matmul(
                        st[:, QW:2 * QW],
                        lhsT=kT[64:128, 128 * kb:128 * (kb + 1)],
                        rhs=qT[64:128, QW * qs:QW * (qs + 1)],
                        start=True, stop=True,
                    )
                    sts[kb] = st

                def emit_o(kb):
                    pt = pts[kb]
                    nc.tensor.matmul(
                        ot_a,
                        lhsT=v_t[kb][:, 2 * p, :],
                        rhs=pt[:, 0:QW],
                        start=(kb == 0), stop=(kb == TOK - 1),
                    )
                    nc.tensor.matmul(
                        ot_b,
                        lhsT=v_t[kb][:, 2 * p + 1, :],
                        rhs=pt[:, QW:2 * QW],
                        start=(kb == 0), stop=(kb == TOK - 1),
                    )

                emit_s(0)
                for kb in range(TOK):
                    pt = ptpool.tile([128, 2 * QW], BF16, name="pt", tag="pt")
                    nc.scalar.activation(out=pt, in_=sts[kb], func=EXP,
                                         scale=0.125)
                    pts[kb] = pt
                    if kb + 1 < TOK:
                        emit_s(kb + 1)
                    if kb >= OTRAIL:
                        emit_o(kb - OTRAIL)
                    pop_fillers(s, kb, BUDGET[s])
                for kb in range(TOK - OTRAIL, TOK):
                    emit_o(kb)

                # Copy psum out fast (frees the O banks), then invert the
                # denominators.  The 1024 dens live in ONE partition; a
                # 1->64 broadcast DMA from a single partition serializes
                # ~256KB on that partition's port (~10us!), so instead:
                # scatter them across 128 partitions (4KB, one port read),
                # reciprocal on all 128 lanes (8 elems/lane), then broadcast
                # reading only 32B per partition from all 128 ports.
                oc = rpool.tile([65, 2 * QW], F32, name="oc", tag="oc")
                nc.vector.tensor_copy(out=oc[:, 0:QW], in_=ot_a)
                nc.vector.tensor_copy(out=oc[:, QW:2 * QW], in_=ot_b)
                PW = 2 * QW // 128
                # the normalize chain's small DMAs must not wait on the big
                # out-store DMAs: HWDGE completion semaphores share 8 lanes,
                # so a 4KB scatter behind a 256KB store inherits its latency.
                # SWDGE (gpsimd) uses the separate DMA-SW lanes and an
                # otherwise-idle queue.
                dmae = nc.gpsimd
                rp = rpool if p == 0 else rpool2
                rec_t = rp.tile([128, PW], F32, name="rec_t", tag="rec_t")
                ocstep = oc.ap[0][0]
                dmae.dma_start(out=rec_t, in_=bass.AP(
                    tensor=oc.tensor, offset=oc.offset + 64 * ocstep,
                    ap=[[ocstep, 1], [PW, 128], [1, PW]]))
                rec_r = rp.tile([128, PW], F32, name="rec_r", tag="rec_r")
                nc.vector.reciprocal(out=rec_r, in_=rec_t)
                # unscatter to a row, then tree-broadcast 1->8->64 (SBUF free
                # dims cannot cross partitions, and a direct 1->64 broadcast
                # serializes ~256KB on the source partition's port)
                rrow = rp.tile([8, 2 * QW], F32, name="rrow", tag="rrow")
                rowstep = rrow.ap[0][0]
                rstep = rec_r.ap[0][0]
                dmae.dma_start(
                    out=bass.AP(tensor=rrow.tensor, offset=rrow.offset,
                                ap=[[rowstep, 1], [PW, 128], [1, PW]]),
                    in_=bass.AP(tensor=rec_r.tensor, offset=rec_r.offset,
                                ap=[[rstep, 128], [1, PW]]))
                dmae.dma_start(
                    out=bass.AP(tensor=rrow.tensor,
                                offset=rrow.offset + rowstep,
                                ap=[[rowstep, 7], [1, 2 * QW]]),
                    in_=bass.AP(tensor=rrow.tensor, offset=rrow.offset,
                                ap=[[rowstep, 1], [0, 7], [1, 2 * QW]]))
                rbc = rpool.tile([64, 2 * QW], F32, name="rbc", tag="rbc")
                dmae.dma_start(
                    out=rbc,
                    in_=bass.AP(tensor=rrow.tensor, offset=rrow.offset,
                                ap=[[rowstep, 8], [0, 8], [1, 2 * QW]]))
                nc.gpsimd.tensor_mul(
                    out=ot_sb[p][qs][0:64, :],
                    in0=oc[0:64, 0:QW], in1=rbc[:, 0:QW])
                stg = rpool.tile([64, QW], BF16, name="stg", tag="stg")
                nc.gpsimd.tensor_mul(
                    out=stg, in0=oc[0:64, QW:2 * QW], in1=rbc[:, QW:2 * QW])
                # engines are lane-aligned; a DMA moves the odd head into
                # partitions 64-127 so the out-proj can contract K=128
                dmae.dma_start(out=ot_sb[p][qs][64:128, :], in_=stg)

            # drain any fillers not yet finished, then the last proj chunk
            while fillers:
                try:
                    next(fillers[0][1])
                except StopIteration:
                    fillers.pop(0)
            for t in range(12, TOK):
                for _ in gen_proj(t, last=True, pairs=(1,)):
                    pass

    nc.compile()
    return nc


def _get_nc():
    if "nc" not in _cache:
        _cache["nc"] = _build_nc()
    return _cache["nc"]


def kernel(x, w_qkv, w_proj, b_proj):
    import ml_dtypes
    from concourse.bass_utils import run_bass_kernel_spmd

    x = np.asarray(x, dtype=np.float32)
    w_qkv = np.asarray(w_qkv, dtype=np.float32)
    w_proj = np.asarray(w_proj, dtype=np.float32)
    b_proj = np.asarray(b_proj, dtype=np.float32)

    nc = _get_nc()
    in_maps = []
    for core in range(NCORES):
        b, g = divmod(core, 4)
        rows = np.concatenate([
            np.arange(C * j + HL * D * g, C * j + HL * D * (g + 1))
            for j in range(3)
        ])
        in_maps.append({
            "xT": np.ascontiguousarray(x[b].T).astype(ml_dtypes.bfloat16),
            "wT": np.ascontiguousarray(w_qkv[rows].T).astype(ml_dtypes.bfloat16),
            "wpT": np.ascontiguousarray(
                w_proj[:, HL * D * g:HL * D * (g + 1)].T).astype(ml_dtypes.bfloat16),
        })

    res = run_bass_kernel_spmd(
        nc, in_maps, list(range(NCORES)),
        trace=bool(os.environ.get("KERNEL_TRACE")),
    )
    _cache["last_results"] = res

    out = np.empty((B, L, C), dtype=np.float32)
    for b in range(B):
        acc = res.results[4 * b]["out"].astype(np.float32)
        acc[L - 2 * QW:] += res.results[4 * b]["out2"]
        for g in range(1, 4):
            acc = acc + res.results[4 * b + g]["out"]
            acc[L - 2 * QW:] += res.results[4 * b + g]["out2"]
        out[b] = acc + b_proj[None, :]
    return out


# revision 31
# speedup vs baseline: 1.0389x; 1.0389x over previous
"""Multi-head attention (B=2, L=2048, dim=1024, 16 heads) on 8 Trainium2 cores.

Sharding: 8 cores = 2 (batch) x 4 (head groups of 4 heads). Each core runs an
identical Bass program on its own slice (SPMD, no collectives); the host sums
the 4 per-head-group partial projection outputs per batch and adds the bias.

Per-core dataflow (bf16 matmul operands, fp32 PSUM accumulation):
  xT [1024, 2048]  (x[b] transposed, channel-major, bf16)
  V token-major [128 tok, 4 heads, 64+1] (ones column fused for the softmax
    denominator), qT/kT feature-major [128 (2 heads x 64d), 2048]
  ST[k, q] = kT.T @ qT    (K=64 contraction, head pairs row-packed in the PE)
  PT = exp(ST / 8)        (ScalarE, PSUM -> SBUF bf16)
  OT[d, q] += V.T @ PT    (M=65: row 64 accumulates the softmax denominator)
  OT_norm = OT * bcast(1/denom), out = OT_norm.T @ wpT

Scheduling: one flat software pipeline paced by the ScalarE exp stream.
x and w_qkv live in single wide SBUF tiles loaded by ONE dma_start per
512-token x wave / per w section (HWDGE issue on the sync sequencer costs
~0.6us per dma_start regardless of size, so consolidation is what gets the
first score matmul gated by only ~2 MB of DMA).  The attention loop is
software-pipelined (scores k+1 issue right after exp k; O-matmuls trail by 4
steps) and all other projection work is chopped into ~2-matmul micro-units
fed between attention steps at a budgeted rate, keeping the PE dense (no HAM
re-throttle) without pushing the next score matmul back by more than ~0.5us.
The per-(pair,qs) normalized outputs live in separate tiles so the out-proj
fillers never false-share with the current super's normalize writes, and the
softmax reciprocal uses the fast custom-DVE approximation (~51 ULP, fine for
a softmax denominator) instead of the ~6.5us iterative-divide reciprocal.
"""

import os
import numpy as np

B, L, C = 2, 2048, 1024
H, D = 16, 64
HL = 4            # heads per core (local)
PAIRS = 2         # head pairs per core
CT = C // 128     # 8 contraction tiles for the projections
TOK = L // 128    # 16 key-token tiles
QW = 512          # query tile width
QS = L // QW      # 4 query tiles
NCORES = 8
OTRAIL = 4        # O-accumulation matmuls trail the exp stream by this many kb

_cache = {}


def _build_nc():
    import concourse.bass as bass
    import concourse.mybir as mybir
    import concourse.tile as tile
    from concourse import bacc

    F32 = mybir.dt.float32
    BF16 = mybir.dt.bfloat16
    EXP = mybir.ActivationFunctionType.Exp

    nc = bacc.Bacc("TRN2", target_bir_lowering=False, debug=False,
                   num_devices=NCORES)

    xT = nc.declare_dram_parameter("xT", [C, L], BF16, isOutput=False)
    wT = nc.declare_dram_parameter("wT", [C, 3 * HL * D], BF16, isOutput=False)
    wpT = nc.declare_dram_parameter("wpT", [HL * D, C], BF16, isOutput=False)
    out = nc.declare_dram_parameter("out", [L, C], BF16, isOutput=True)
    # pair-0 half of the final out-proj chunk, streamed out early; the host
    # adds it into rows [L-QW, L) (the device-side add would serialize the
    # kernel tail behind the very last normalize)
    out2 = nc.declare_dram_parameter("out2", [2 * QW, C], BF16, isOutput=True)

    xTr = xT.rearrange("(c p) l -> p c l", p=128)
    wTr = wT.rearrange("(c p) m -> p c m", p=128)

    with tile.TileContext(nc) as tc:
        from contextlib import ExitStack
        with ExitStack() as ctx:
            xpool = ctx.enter_context(tc.tile_pool(name="x", bufs=1))
            wpool = ctx.enter_context(tc.tile_pool(name="w", bufs=1))
            wppool = ctx.enter_context(tc.tile_pool(name="wp", bufs=1))
            qkpool = ctx.enter_context(tc.tile_pool(name="qk", bufs=1))
            vpool = ctx.enter_context(tc.tile_pool(name="v", bufs=1))
            psS = ctx.enter_context(tc.tile_pool(name="psS", bufs=2, space="PSUM"))
            psO = ctx.enter_context(tc.tile_pool(name="psO", bufs=4, space="PSUM"))
            otpool = ctx.enter_context(tc.tile_pool(name="ot", bufs=1))
            ptpool = ctx.enter_context(tc.tile_pool(name="pt", bufs=8))
            rpool = ctx.enter_context(tc.tile_pool(name="r", bufs=2))
            rpool2 = ctx.enter_context(tc.tile_pool(name="r2", bufs=2))
            obpool = ctx.enter_context(tc.tile_pool(name="ob", bufs=4))

            # ---- warmup: exp table load + a PE matmul burst at t=0 so the
            # HAM clock gate is at full rate before the first projection
            warm = rpool.tile([128, 16], F32, name="warm", tag="warm")
            nc.vector.memset(warm[:, 0:8], 0.0)
            nc.scalar.activation(out=warm[:, 8:16], in_=warm[:, 0:8], func=EXP)
            warmb = rpool.tile([128, 512], BF16, name="warmb", tag="warmb")
            nc.vector.memset(warmb, 0.0)
            wps = psO.tile([64, 512], F32, name="warmps", tag="ot")
            for i in range(24):
                nc.tensor.matmul(wps, lhsT=warmb[:, 0:64], rhs=warmb,
                                 start=(i == 0), stop=(i == 23))

            # ---- input loads: one dma_start per x wave / w section ----------
            x_b = xpool.tile([128, CT, L], BF16, name="x", tag="x")
            w_b = wpool.tile([128, CT, 3 * HL * D], BF16, name="w", tag="w")

            def wsec(j):  # j: 0=q, 1=k, 2=v section of w_qkv
                nc.sync.dma_start(
                    out=w_b[:, :, j * HL * D:(j + 1) * HL * D],
                    in_=wTr[:, :, j * HL * D:(j + 1) * HL * D])

            def xwave(ns):
                nc.sync.dma_start(
                    out=x_b[:, :, QW * ns:QW * (ns + 1)],
                    in_=xTr[:, :, QW * ns:QW * (ns + 1)])

            # q+k sections are contiguous in w -> one DMA covers both
            nc.sync.dma_start(out=w_b[:, :, 0:2 * HL * D],
                              in_=wTr[:, :, 0:2 * HL * D])
            xwave(0)
            xwave(1)
            wsec(2)
            xwave(2)
            xwave(3)
            wp_t = []
            for p in range(PAIRS):
                t = wppool.tile([128, C], BF16, name=f"wp{p}", tag=f"wp{p}")
                nc.sync.dma_start(out=t, in_=wpT[2 * D * p:2 * D * (p + 1), :])
                wp_t.append(t)

            ones_s = vpool.tile([128, HL, 1], F32, name="ones_s", tag="ones_s")
            nc.vector.memset(ones_s, 1.0)

            # ---- projection building blocks (filler generators) -------------
            v_t = [None] * TOK

            def gen_v(t):
                ps = psO.tile([128, HL * D], F32, name="psv", tag="ot")
                for c0 in range(0, CT, 2):
                    for c in (c0, c0 + 1):
                        nc.tensor.matmul(
                            ps,
                            lhsT=x_b[:, c, 128 * t:128 * (t + 1)],
                            rhs=w_b[:, c, 2 * HL * D:3 * HL * D],
                            start=(c == 0), stop=(c == CT - 1),
                        )
                    yield
                vt = vpool.tile([128, HL, D + 1], BF16, name=f"v{t}", tag=f"v{t}")
                nc.vector.tensor_copy(out=vt[:, :, D:D + 1], in_=ones_s)
                nc.vector.tensor_copy(
                    out=vt[:, :, 0:D],
                    in_=ps.rearrange("p (h d) -> p h d", h=HL),
                )
                v_t[t] = vt
                yield

            qk_t = {}
            for p in range(PAIRS):
                for nm in ("q", "k"):
                    qk_t[(nm, p)] = qkpool.tile(
                        [128, L], BF16, name=f"{nm}{p}", tag=f"{nm}{p}")

            def gen_qk(nm, p, ns):
                j = 0 if nm == "q" else 1
                ps = psO.tile([128, QW], F32, name="psqk", tag="ot")
                for c0 in range(0, CT, 2):
                    for c in (c0, c0 + 1):
                        nc.tensor.matmul(
                            ps,
                            lhsT=w_b[:, c, j * HL * D + 128 * p:
                                     j * HL * D + 128 * (p + 1)],
                            rhs=x_b[:, c, QW * ns:QW * (ns + 1)],
                            start=(c == 0), stop=(c == CT - 1),
                        )
                    yield
                nc.vector.tensor_copy(
                    out=qk_t[(nm, p)][:, QW * ns:QW * (ns + 1)], in_=ps)
                yield

            # normalized attention outputs, one tile per (pair, qs) so the
            # out-proj fillers never false-share with later normalize writes
            ot_sb = [[otpool.tile([128, QW], BF16, name=f"otp{p}q{qs}",
                                  tag=f"otp{p}q{qs}")
                      for qs in range(QS)] for p in range(PAIRS)]

            def gen_proj(t, last=False, pairs=tuple(range(PAIRS)), dst=None,
                         dst_row=None):
                qs, tq = divmod(t, QW // 128)
                ob = obpool.tile([128, C], BF16, name="ob", tag="ob")
                if dst is None:
                    dst, dst_row = out, 128 * t
                for nh in range(C // QW):
                    ps = psO.tile([128, QW], F32, name="psp", tag="ot")
                    for j, p2 in enumerate(pairs):
                        nc.tensor.matmul(
                            ps,
                            lhsT=ot_sb[p2][qs][:, 128 * tq:128 * (tq + 1)],
                            rhs=wp_t[p2][:, QW * nh:QW * (nh + 1)],
                            start=(j == 0), stop=(j == len(pairs) - 1),
                        )
                    yield
                    if last and nh == 0:
                        # tail: ACT is idle by now, offload one copy there
                        nc.scalar.activation(
                            out=ob[:, QW * nh:QW * (nh + 1)], in_=ps,
                            func=mybir.ActivationFunctionType.Copy)
                    else:
                        nc.vector.tensor_copy(
                            out=ob[:, QW * nh:QW * (nh + 1)], in_=ps)
                    yield
                nc.sync.dma_start(out=dst[dst_row:dst_row + 128, :], in_=ob)

            # ---- prologue: only what gates the first exp ---------------------
            for _ in gen_qk("k", 0, 0):
                pass
            for _ in gen_qk("q", 0, 0):
                pass

            # ---- filler queue ------------------------------------------------
            # (gate, generator): generator may be advanced once the super
            # with index >= gate is being emitted.  Supers are p-major:
            # s = 4*p + qs.
            fillers = []

            def F(gate, g, min_kb=0):
                fillers.append((gate, min_kb, g))

            F(0, gen_qk("k", 0, 1))
            for t in range(0, 3):
                F(0, gen_v(t))
            F(0, gen_qk("k", 0, 2))
            for t in range(3, 6):
                F(0, gen_v(t))
            F(0, gen_qk("k", 0, 3))
            for t in range(6, 9):
                F(0, gen_v(t))
            F(0, gen_qk("q", 0, 1))
            for t in range(9, TOK):
                F(0, gen_v(t))
            for ns in range(QS):
                F(1, gen_qk("k", 1, ns))
            F(1, gen_qk("q", 1, 0))
            F(1, gen_qk("q", 0, 2))
            F(2, gen_qk("q", 0, 3))
            # proj units read the PREVIOUS super's normalize output, which
            # completes a few us into the gating super -- hold them until
            # mid-super so they never block the in-order PE stream
            for t in range(8, 12):
                F(3, gen_proj(t, pairs=(0,), dst=out2,
                              dst_row=128 * (t - 8)), min_kb=7)
            F(4, gen_qk("q", 1, 1))
            for t in range(12, TOK):
                F(4, gen_proj(t, pairs=(0,), dst=out2,
                              dst_row=QW + 128 * (t - 12)), min_kb=7)
            F(5, gen_qk("q", 1, 2))
            for t in range(0, 4):
                F(5, gen_proj(t), min_kb=7)
            F(6, gen_qk("q", 1, 3))
            for t in range(4, 8):
                F(6, gen_proj(t), min_kb=7)
            for t in range(8, 12):
                F(7, gen_proj(t, pairs=(1,)), min_kb=7)
            BUDGET = [8, 3, 2, 3, 3, 3, 3, 3]

            def pop_fillers(s, kb, budget):
                popped = 0
                while popped < budget and fillers:
                    gate, min_kb, g = fillers[0]
                    if gate > s or (gate == s and kb < min_kb):
                        break
                    try:
                        next(g)
                        popped += 1
                    except StopIteration:
                        fillers.pop(0)

            # ---- attention ---------------------------------------------------
            # software-pipelined: scores k+1 issue right after exp k; the
            # O-accumulation matmuls (which depend on exp output) trail by
            # OTRAIL steps so the PE never stalls waiting on ScalarE.
            for s in range(PAIRS * QS):
                p, qs = divmod(s, QS)
                kT = qk_t[("k", p)]
                qT = qk_t[("q", p)]
                ot_a = psO.tile([65, QW], F32, name="ot_a", tag="ot")
                ot_b = psO.tile([65, QW], F32, name="ot_b", tag="ot")
                sts = [None] * TOK
                pts = [None] * TOK

                def emit_s(kb):
                    st = psS.tile([128, 2 * QW], F32, name="st", tag="st")
                    nc.tensor.matmul(
                        st[:, 0:QW],
                        lhsT=kT[0:64, 128 * kb:128 * (kb + 1)],
                        rhs=qT[0:64, QW * qs:QW * (qs + 1)],
                        start=True, stop=True,
                    )
                    nc.tensor.matmul(
                        st[:, QW:2 * QW],
                        lhsT=kT[64:128, 128 * kb:128 * (kb + 1)],
                        rhs=qT[64:128, QW * qs:QW * (qs + 1)],
                        start=True, stop=True,
                    )
                    sts[kb] = st

                def emit_o(kb):
                    pt = pts[kb]
                    nc.tensor.matmul(
                        ot_a,
                        lhsT=v_t[kb][:, 2 * p, :],
                        rhs=pt[:, 0:QW],
                        start=(kb == 0), stop=(kb == TOK - 1),
                    )
                    nc.tensor.matmul(
                        ot_b,
                        lhsT=v_t[kb][:, 2 * p + 1, :],
                        rhs=pt[:, QW:2 * QW],
                        start=(kb == 0), stop=(kb == TOK - 1),
                    )

                emit_s(0)
                for kb in range(TOK):
                    pt = ptpool.tile([128, 2 * QW], BF16, name="pt", tag="pt")
                    nc.scalar.activation(out=pt, in_=sts[kb], func=EXP,
                                         scale=0.125)
                    pts[kb] = pt
                    if kb + 1 < TOK:
                        emit_s(kb + 1)
                    if kb >= OTRAIL:
                        emit_o(kb - OTRAIL)
                    pop_fillers(s, kb, BUDGET[s])
                for kb in range(TOK - OTRAIL, TOK):
                    emit_o(kb)

                # Copy psum out fast (frees the O banks), then invert the
                # denominators.  The 1024 dens live in ONE partition; a
                # 1->64 broadcast DMA from a single partition serializes
                # ~256KB on that partition's port (~10us!), so instead:
                # scatter them across 128 partitions (4KB, one port read),
                # reciprocal on all 128 lanes (8 elems/lane), then broadcast
                # reading only 32B per partition from all 128 ports.
                oc = rpool.tile([65, 2 * QW], F32, name="oc", tag="oc")
                nc.vector.tensor_copy(out=oc[:, 0:QW], in_=ot_a)
                nc.vector.tensor_copy(out=oc[:, QW:2 * QW], in_=ot_b)
                PW = 2 * QW // 128
                # the normalize chain's small DMAs must not wait on the big
                # out-store DMAs: HWDGE completion semaphores share 8 lanes,
                # so a 4KB scatter behind a 256KB store inherits its latency.
                # SWDGE (gpsimd) uses the separate DMA-SW lanes and an
                # otherwise-idle queue.
                dmae = nc.gpsimd if s == PAIRS * QS - 1 else nc.sync
                rp = rpool if p == 0 else rpool2
                rec_t = rp.tile([128, PW], F32, name="rec_t", tag="rec_t")
                ocstep = oc.ap[0][0]
                dmae.dma_start(out=rec_t, in_=bass.AP(
                    tensor=oc.tensor, offset=oc.offset + 64 * ocstep,
                    ap=[[ocstep, 1], [PW, 128], [1, PW]]))
                rec_r = rp.tile([128, PW], F32, name="rec_r", tag="rec_r")
                nc.vector.reciprocal(out=rec_r, in_=rec_t)
                # unscatter to a row, then tree-broadcast 1->8->64 (SBUF free
                # dims cannot cross partitions, and a direct 1->64 broadcast
                # serializes ~256KB on the source partition's port)
                rrow = rp.tile([8, 2 * QW], F32, name="rrow", tag="rrow")
                rowstep = rrow.ap[0][0]
                rstep = rec_r.ap[0][0]
                dmae.dma_start(
                    out=bass.AP(tensor=rrow.tensor, offset=rrow.offset,
                                ap=[[rowstep, 1], [PW, 128], [1, PW]]),
                    in_=bass.AP(tensor=rec_r.tensor, offset=rec_r.offset,
                                ap=[[rstep, 128], [1, PW]]))
                dmae.dma_start(
                    out=bass.AP(tensor=rrow.tensor,
                                offset=rrow.offset + rowstep,
                                ap=[[rowstep, 7], [1, 2 * QW]]),
                    in_=bass.AP(tensor=rrow.tensor, offset=rrow.offset,
                                ap=[[rowstep, 1], [0, 7], [1, 2 * QW]]))
                rbc = rpool.tile([64, 2 * QW], F32, name="rbc", tag="rbc")
                dmae.dma_start(
                    out=rbc,
                    in_=bass.AP(tensor=rrow.tensor, offset=rrow.offset,
                                ap=[[rowstep, 8], [0, 8], [1, 2 * QW]]))
                nc.gpsimd.tensor_mul(
                    out=ot_sb[p][qs][0:64, :],
                    in0=oc[0:64, 0:QW], in1=rbc[:, 0:QW])
                stg = rpool.tile([64, QW], BF16, name="stg", tag="stg")
                nc.gpsimd.tensor_mul(
                    out=stg, in0=oc[0:64, QW:2 * QW], in1=rbc[:, QW:2 * QW])
                # engines are lane-aligned; a DMA moves the odd head into
                # partitions 64-127 so the out-proj can contract K=128
                dmae.dma_start(out=ot_sb[p][qs][64:128, :], in_=stg)

            # drain any fillers not yet finished, then the last proj chunk
            while fillers:
                try:
                    next(fillers[0][1])
                except StopIteration:
                    fillers.pop(0)
            for t in range(12, TOK):
                for _ in gen_proj(t, last=True, pairs=(1,)):
                    pass

    nc.compile()
    return nc


def _get_nc():
    if "nc" not in _cache:
        _cache["nc"] = _build_nc()
    return _cache["nc"]


def kernel(x, w_qkv, w_proj, b_proj):
    import ml_dtypes
    from concourse.bass_utils import run_bass_kernel_spmd

    x = np.asarray(x, dtype=np.float32)
    w_qkv = np.asarray(w_qkv, dtype=np.float32)
    w_proj = np.asarray(w_proj, dtype=np.float32)
    b_proj = np.asarray(b_proj, dtype=np.float32)

    nc = _get_nc()
    in_maps = []
    for core in range(NCORES):
        b, g = divmod(core, 4)
        rows = np.concatenate([
            np.arange(C * j + HL * D * g, C * j + HL * D * (g + 1))
            for j in range(3)
        ])
        in_maps.append({
            "xT": np.ascontiguousarray(x[b].T).astype(ml_dtypes.bfloat16),
            "wT": np.ascontiguousarray(w_qkv[rows].T).astype(ml_dtypes.bfloat16),
            "wpT": np.ascontiguousarray(
                w_proj[:, HL * D * g:HL * D * (g + 1)].T).astype(ml_dtypes.bfloat16),
        })

    res = run_bass_kernel_spmd(
        nc, in_maps, list(range(NCORES)),
        trace=bool(os.environ.get("KERNEL_TRACE")),
    )
    _cache["last_results"] = res

    out = np.empty((B, L, C), dtype=np.float32)
    for b in range(B):
        acc = res.results[4 * b]["out"].astype(np.float32)
        acc[L - 2 * QW:] += res.results[4 * b]["out2"]
        for g in range(1, 4):
            acc = acc + res.results[4 * b + g]["out"]
            acc[L - 2 * QW:] += res.results[4 * b + g]["out2"]
        out[b] = acc + b_proj[None, :]
    return out
